# revision 7
# baseline (speedup 1.0000x reference)
"""CondNet kernel for Trainium2 (8 NeuronCores, model-parallel mid layers).

v2 over v1 (v1 measured 491 us, of which 232 us HAM-throttled PE):
  - DMA issue split across both HWDGE rings: the 16.8 MB/layer S-slab
    stream goes on the SP (sync) ring, the 8.4 MB/layer activation
    stream on the Activation (scalar) ring, bounce-out copies and
    collectives on gpsimd (SWDGE).  v1 pushed everything through the
    sync ring, whose FIFO sequencing starved the PE every k-tile and
    kept re-triggering the HAM throttle (50% util cap).
  - Streams fetch 2 k-tiles per DMA (512 KB / 256 KB) to halve the
    per-DMA fixed costs.
  - Each AllGather is split into two 0.5 MB halves: the first half
    fires while the producing layer's second half is still computing
    (S rows are host-permuted to the half-gathered k-order).
  - No third AllGather: L4 (out = h3 @ W_out.T) is computed as
    per-core k-partials straight from L3's SBUF activation tiles and
    the 8 partials are summed on the host.
  - As v1: condensed layers as dense h @ S matmuls (S built on host
    from indx_seqs/W_mid), k-major activations, S slabs stationary /
    activations moving, bf16 data with fp32 PSUM accumulation,
    bias+ReLU fused into the Scalar-engine PSUM eviction.
"""

import os
import numpy as np
import ml_dtypes

import concourse.bass as bass
import concourse.tile as tile
from concourse import bacc, mybir
from concourse import bass_utils

BF16 = ml_dtypes.bfloat16

B, NUM_IN, NUM_MID, NUM_OUT, FAN_IN, N_COND = 512, 1024, 8192, 1000, 64, 2
NCORES = 8
OSLAB = NUM_MID // NCORES      # 1024 outputs per core for mid layers
HALF = OSLAB // 2              # 512-row AllGather halves
RG = [list(range(NCORES))]     # one replica group: all 8 cores

_cache = {}
LAST_RESULT = None  # BassKernelResults of the most recent run (for test harness)


def _build_nc():
    """Build + compile the Bass program (same SPMD program for all 8 cores)."""
    nc = bacc.Bacc("TRN2", target_bir_lowering=False, debug=False, num_devices=NCORES)
    f32, bf16 = mybir.dt.float32, mybir.dt.bfloat16

    # ---- DRAM I/O (per-core slabs; k-major tiles everywhere) ----
    xT_d = nc.dram_tensor("xT", [8, 128, B], bf16, kind="ExternalInput").ap()
    w_in_d = nc.dram_tensor("w_in", [8, 128, OSLAB], bf16, kind="ExternalInput").ap()
    b_in_d = nc.dram_tensor("b_in", [128, 8], f32, kind="ExternalInput").ap()
    s1_d = nc.dram_tensor("s1", [64, 128, OSLAB], bf16, kind="ExternalInput").ap()
    b1_d = nc.dram_tensor("b1", [128, 8], f32, kind="ExternalInput").ap()
    s2_d = nc.dram_tensor("s2", [64, 128, OSLAB], bf16, kind="ExternalInput").ap()
    b2_d = nc.dram_tensor("b2", [128, 8], f32, kind="ExternalInput").ap()
    w_out_d = nc.dram_tensor("w_out", [8, 128, 1024], bf16, kind="ExternalInput").ap()
    out_d = nc.dram_tensor("out", [8, 128, B], f32, kind="ExternalOutput").ap()

    with tile.TileContext(nc) as tc:
        with (
            tc.tile_pool(name="const", bufs=1) as cpool,
            tc.tile_pool(name="sstream", bufs=10) as spool,
            tc.tile_pool(name="hstream", bufs=10) as hpool,
            tc.tile_pool(name="acts", bufs=4) as apool,
            tc.tile_pool(name="psmm", bufs=1, space="PSUM") as pmm,
            tc.tile_pool(name="dram", bufs=1, space="DRAM") as dpool,
        ):
            # persistent SBUF tensors. xT/w_in on the sync ring (needed
            # first), w_out/biases on the scalar ring (idle at start).
            xT = cpool.tile([128, 8, B], bf16)
            nc.sync.dma_start(out=xT[:], in_=xT_d.rearrange("u p b -> p u b"))
            w_in = cpool.tile([128, 8, OSLAB], bf16)
            nc.sync.dma_start(out=w_in[:], in_=w_in_d.rearrange("u p o -> p u o"))
            w_out = cpool.tile([128, 8, 1024], bf16)
            nc.scalar.dma_start(out=w_out[:], in_=w_out_d.rearrange("u p j -> p u j"))
            b_in = cpool.tile([128, 8], f32)
            b1 = cpool.tile([128, 8], f32)
            b2 = cpool.tile([128, 8], f32)
            nc.scalar.dma_start(out=b_in[:], in_=b_in_d)
            nc.scalar.dma_start(out=b1[:], in_=b1_d)
            nc.scalar.dma_start(out=b2[:], in_=b2_d)

            # DRAM bounce buffers: per-half AllGathers (tile-major 3D so
            # sliced chunks can be partition-rearranged on the fly)
            h1b = [dpool.tile([4, 128, B], bf16, tag=f"h1b{i}", name=f"h1b{i}")
                   for i in range(2)]
            h1g = [dpool.tile([32, 128, B], bf16, tag=f"h1g{i}",
                              name=f"h1g{i}", addr_space="Shared")
                   for i in range(2)]
            h2b = [dpool.tile([4, 128, B], bf16, tag=f"h2b{i}", name=f"h2b{i}")
                   for i in range(2)]
            h2g = [dpool.tile([32, 128, B], bf16, tag=f"h2g{i}",
                              name=f"h2g{i}", addr_space="Shared")
                   for i in range(2)]

            def evict(psums, bias, bounce, keep_sbuf=False):
                """PSUM -> bias+ReLU -> SBUF bf16 -> (DRAM half-bounces + AG).

                Emits the half-AllGather right after each half's 4 tiles
                are out.  Returns the SBUF act tiles if keep_sbuf.
                """
                kept = []
                for half in range(2):
                    for i in range(4):
                        ot = half * 4 + i
                        if keep_sbuf:
                            act = cpool.tile([128, B], bf16, tag=f"h3k{ot}",
                                             name=f"h3k{ot}")
                        else:
                            act = apool.tile([128, B], bf16, tag="act",
                                             name="act")
                        nc.scalar.activation(
                            act[:], psums[ot][:],
                            mybir.ActivationFunctionType.Relu,
                            bias=bias[:, ot:ot + 1],
                        )
                        kept.append(act)
                        if bounce is not None:
                            nc.gpsimd.dma_start(
                                out=bounce[0][half][i], in_=act[:])
                    if bounce is not None:
                        nc.gpsimd.collective_compute(
                            "AllGather", mybir.AluOpType.bypass,
                            replica_groups=RG,
                            ins=[bounce[0][half].opt()],
                            outs=[bounce[1][half].opt()])
                return kept if keep_sbuf else None

            def mid_layer(h_halves, s_dram, bias, bounce, keep_sbuf=False):
                """Full-batch dense layer: out slab (1024 cols) of h @ S.

                h_halves: two DRAM [32, 128, 512] bf16 gathered halves
                          (k-order: S rows are host-permuted to match)
                """
                psums = [pmm.tile([128, B], f32, tag=f"mm{ot}", name=f"ps{ot}")
                         for ot in range(8)]
                for kt in range(64):
                    se = nc.sync if kt % 2 == 0 else nc.scalar
                    he = nc.scalar if kt % 2 == 0 else nc.sync
                    st = spool.tile([128, OSLAB], bf16, tag="s", name="st")
                    se.dma_start(out=st[:], in_=s_dram[kt])
                    ht = hpool.tile([128, B], bf16, tag="h", name="ht")
                    he.dma_start(out=ht[:], in_=h_halves[kt // 32][kt % 32])
                    for ot in range(8):
                        nc.tensor.matmul(
                            psums[ot][:],
                            st[:, ot * 128:(ot + 1) * 128],
                            ht[:],
                            start=(kt == 0),
                            stop=(kt == 63),
                        )
                return evict(psums, bias, bounce, keep_sbuf)

            # ---- L1: h1 slab = relu(W_in_slab.T @ xT + b_in) ----
            psums = [pmm.tile([128, B], f32, tag=f"mm{ot}", name=f"ps{ot}")
                     for ot in range(8)]
            for kt in range(8):
                for ot in range(8):
                    nc.tensor.matmul(
                        psums[ot][:],
                        w_in[:, kt, ot * 128:(ot + 1) * 128],
                        xT[:, kt, :],
                        start=(kt == 0),
                        stop=(kt == 7),
                    )
            evict(psums, b_in, (h1b, h1g))

            mid_layer(h1g, s1_d, b1, (h2b, h2g))
            h3 = mid_layer(h2g, s2_d, b2, None, keep_sbuf=True)

            # ---- L4: per-core k-partial of out = W_out_slab.T @ h3_slab ----
            psums = [pmm.tile([128, B], f32, tag=f"mm{jt}", name=f"po{jt}")
                     for jt in range(8)]
            for kt in range(8):
                for jt in range(8):
                    nc.tensor.matmul(
                        psums[jt][:],
                        w_out[:, kt, jt * 128:(jt + 1) * 128],
                        h3[kt][:],
                        start=(kt == 0),
                        stop=(kt == 7),
                    )
            for jt in range(8):
                osb = apool.tile([128, B], f32, tag="out", name="osb")
                nc.vector.tensor_copy(osb[:], psums[jt][:])
                nc.gpsimd.dma_start(out=out_d[jt], in_=osb[:])

    nc.compile()
    return nc


def _perm():
    """k-order of the half-gathered activations: rank-major halves."""
    return np.concatenate(
        [np.arange(r * OSLAB + h * HALF, r * OSLAB + (h + 1) * HALF)
         for h in range(2) for r in range(NCORES)])


def _prep_inputs(x, W_in, b_in, W_mid, b_mid, W_out, b_out, indx_seqs):
    """Host-side compile-time transforms of inputs (per-core slabs)."""
    idx = np.asarray(indx_seqs).astype(np.int64)
    perm = _perm()

    def build_S(Wm):
        # S[k, o] = sum_f Wm[o, f] * [idx[o, f] == k], k rows permuted
        S = np.zeros((NUM_MID, NUM_MID), np.float32)
        cols = np.repeat(np.arange(NUM_MID), FAN_IN)
        np.add.at(S, (idx.reshape(-1), cols), np.asarray(Wm, np.float32).reshape(-1))
        return S[perm].reshape(64, 128, NUM_MID).astype(BF16)

    s1_t = build_S(W_mid[0])
    s2_t = build_S(W_mid[1])

    x = np.asarray(x, np.float32)
    xT = np.ascontiguousarray(x.T.reshape(8, 128, B).astype(BF16))
    w_in_t = np.asarray(W_in, np.float32).T.reshape(8, 128, NUM_MID).astype(BF16)
    woT = np.asarray(W_out, np.float32).T  # [8192, 1000]

    def bias_slab(b, c):
        return np.ascontiguousarray(
            np.asarray(b, np.float32)[c * OSLAB:(c + 1) * OSLAB].reshape(8, 128).T)

    in_maps = []
    for c in range(NCORES):
        sl = slice(c * OSLAB, (c + 1) * OSLAB)
        wo = np.zeros((OSLAB, 1024), np.float32)
        wo[:, :NUM_OUT] = woT[sl]
        in_maps.append({
            "xT": xT,
            "w_in": np.ascontiguousarray(w_in_t[:, :, sl]),
            "b_in": bias_slab(b_in, c),
            "s1": np.ascontiguousarray(s1_t[:, :, sl]),
            "b1": bias_slab(b_mid[0], c),
            "s2": np.ascontiguousarray(s2_t[:, :, sl]),
            "b2": bias_slab(b_mid[1], c),
            "w_out": np.ascontiguousarray(wo.reshape(8, 128, 1024).astype(BF16)),
        })
    return in_maps, np.asarray(b_out, np.float32)


def kernel(x, W_in, b_in, W_mid, b_mid, W_out, b_out, indx_seqs):
    global LAST_RESULT
    if "nc" not in _cache:
        _cache["nc"] = _build_nc()
    nc = _cache["nc"]

    in_maps, b_out_f = _prep_inputs(x, W_in, b_in, W_mid, b_mid, W_out, b_out,
                                    indx_seqs)

    res = bass_utils.run_bass_kernel_spmd(
        nc, in_maps, core_ids=list(range(NCORES)),
        trace=bool(int(os.environ.get("KERNEL_TRACE", "0"))),
    )
    LAST_RESULT = res

    acc = np.zeros((1024, B), np.float64)
    for r in res.results:
        acc += r["out"].reshape(1024, B)
    out = acc[:NUM_OUT].T + b_out_f[None, :]
    return np.ascontiguousarray(out).astype(np.float32)
